# revision 8
# baseline (speedup 1.0000x reference)
"""Trainium2 Bass kernel for nn_AttentionBlockPersistent.

Computation (per batch b of 32):
  x0   = concat([x[b] (128ch), pos (32ch)])                 # [160, 1024]
  q    = relu(W_q @ x0 + b_q)                               # [128, 1024]
  f_k  = sigmoid(W_fk @ [x0; k] + b_fk);  c_k = relu(W_ck @ [x0; k] + b_ck)
  k_new = f_k * k + c_k        (same for v)
  S    = q^T K / sqrt(128)     causal-masked softmax rows   # [1024, 1024]
  out  = V @ softmax(S)^T                                   # [128, 1024]

Sharding: data-parallel over batch, 4 batches per core on 8 cores.
Weights/pos/masks replicated. All matmuls run as float32r (full-rate PE);
attention probabilities are computed in transposed layout [m, n] so the
PV matmul needs no transposes of P; the softmax denominator comes from a
ones-vector matmul and is applied via reciprocal + partition_broadcast.
"""

import os
import sys

for _p in ("/opt/trn_rl_repo", os.path.expanduser("~/.axon_site/_ro/trn_rl_repo")):
    if os.path.isdir(_p) and _p not in sys.path:
        sys.path.insert(0, _p)

import numpy as np

import concourse.bass as bass  # noqa: E402
import concourse.mybir as mybir  # noqa: E402
import concourse.tile as tile  # noqa: E402
from concourse import bacc, bass_utils  # noqa: E402

F32 = mybir.dt.float32
F32R = mybir.dt.float32r

N_CORES = 8
B = 32
BPC = B // N_CORES  # batches per core
C = 128
K = 128
V = 128
EMB = 16
SIDE = 32
N = SIDE * SIDE  # 1024
SCALE = 1.0 / float(np.sqrt(K))


# ---------------------------------------------------------------- host prep

def _pos_enc_table(side_len, emb_dim):
    tbl = np.array(
        [
            [p / np.power(10000.0, 2 * (j // 2) / emb_dim) for j in range(emb_dim)]
            if p != 0
            else np.zeros(emb_dim)
            for p in range(side_len)
        ]
    )
    tbl[1:, 0::2] = np.sin(tbl[1:, 0::2])
    tbl[1:, 1::2] = np.cos(tbl[1:, 1::2])
    return tbl.astype(np.float32)


def _pos_embeddings(side_len, emb_dim):
    t = _pos_enc_table(side_len, emb_dim)
    xe = np.broadcast_to(t.reshape(1, emb_dim, side_len, 1), (1, emb_dim, side_len, side_len))
    ye = np.broadcast_to(t.reshape(1, emb_dim, 1, side_len), (1, emb_dim, side_len, side_len))
    pe = np.concatenate([xe, ye], axis=1)
    return np.ascontiguousarray(pe.reshape(2 * emb_dim, side_len * side_len))  # [32, 1024]


# ------------------------------------------------------------- bass program

def build_program(reps: int = 1):
    """Build + compile the per-core program. Returns (nc, input_names)."""
    nc = bacc.Bacc("TRN2", target_bir_lowering=False, debug=False)

    dram_in = {}

    def din(name, shape, dt=F32R):
        t = nc.dram_tensor(name, list(shape), dt, kind="ExternalInput")
        dram_in[name] = t
        return t

    din("x", (BPC, C, N))
    din("k", (BPC, K, N))
    din("v", (BPC, V, N))
    din("pos", (2 * EMB + 1, N))
    din("triu", (128, 128))
    din("ident", (128, 128))
    for nm in ("wq", "wfk", "wck", "wfv", "wcv"):
        din(nm + "_x", (C, K))
        din(nm + "_p", (2 * EMB + 1, K))
        if nm != "wq":
            din(nm + "_k", (K, K))
    din("ones", (128, 1))

    dram_in["o"] = nc.dram_tensor("o", [BPC, V, N], F32, kind="ExternalOutput")
    dram_in["kn"] = nc.dram_tensor("kn", [BPC, K, N], F32, kind="ExternalOutput")
    dram_in["vn"] = nc.dram_tensor("vn", [BPC, V, N], F32, kind="ExternalOutput")

    with tile.TileContext(nc) as tc:
        _emit(tc, nc, dram_in, reps)

    nc.compile()
    in_names = [n for n in dram_in if n not in ("o", "kn", "vn")]
    return nc, in_names


def _emit(tc, nc, d, reps):
    import contextlib

    ctx = contextlib.ExitStack()
    with ctx:
        consts = ctx.enter_context(tc.tile_pool(name="consts", bufs=1))
        io = ctx.enter_context(tc.tile_pool(name="io", bufs=2))
        work = ctx.enter_context(tc.tile_pool(name="work", bufs=2))
        psA = ctx.enter_context(tc.tile_pool(name="psA", bufs=3, space="PSUM"))
        psT = ctx.enter_context(tc.tile_pool(name="psT", bufs=1, space="PSUM"))
        psPV = ctx.enter_context(tc.tile_pool(name="psPV", bufs=2, space="PSUM"))
        psZ = ctx.enter_context(tc.tile_pool(name="psZ", bufs=1, space="PSUM"))

        # ---- load constants into SBUF once
        def cload(name, shape):
            t = consts.tile(list(shape), d[name].dtype, tag=name)
            nc.sync.dma_start(out=t[:], in_=d[name].ap())
            return t

        pos = cload("pos", (2 * EMB + 1, N))
        triu = cload("triu", (128, 128))
        ident = cload("ident", (128, 128))
        wq_x = cload("wq_x", (C, K))
        wq_p = cload("wq_p", (2 * EMB + 1, K))
        wfk_x = cload("wfk_x", (C, K))
        wfk_p = cload("wfk_p", (2 * EMB + 1, K))
        wfk_k = cload("wfk_k", (K, K))
        wck_x = cload("wck_x", (C, K))
        wck_p = cload("wck_p", (2 * EMB + 1, K))
        wck_k = cload("wck_k", (K, K))
        wfv_x = cload("wfv_x", (C, V))
        wfv_p = cload("wfv_p", (2 * EMB + 1, V))
        wfv_v = cload("wfv_k", (V, V))
        wcv_x = cload("wcv_x", (C, V))
        wcv_p = cload("wcv_p", (2 * EMB + 1, V))
        wcv_v = cload("wcv_k", (V, V))
        ones = cload("ones", (128, 1))

        def r(ap):
            return ap

        def body(_i=None):
            for b in range(BPC):
                _emit_batch(tc, nc, d, b, io, work, psA, psT, psPV, psZ, r, pos, triu,
                            ident, ones,
                            (wq_x, wq_p, None),
                            (wfk_x, wfk_p, wfk_k),
                            (wck_x, wck_p, wck_k),
                            (wfv_x, wfv_p, wfv_v),
                            (wcv_x, wcv_p, wcv_v))

        if reps == 1:
            body()
        else:
            with tc.For_i(0, reps, 1) as _i:
                body(_i)


def _emit_batch(tc, nc, d, b, io, work, psA, psT, psPV, psZ, r, pos, triu, ident, ones,
                wq, wfk, wck, wfv, wcv):
    Act = mybir.ActivationFunctionType
    Alu = mybir.AluOpType

    # ---- loads
    xb = io.tile([128, N], F32R, tag="xb")
    kb = io.tile([128, N], F32R, tag="kb")
    vb = io.tile([128, N], F32R, tag="vb")
    nc.sync.dma_start(out=xb[:], in_=d["x"].ap()[b])
    nc.sync.dma_start(out=kb[:], in_=d["k"].ap()[b])
    nc.sync.dma_start(out=vb[:], in_=d["v"].ap()[b])

    # ---- gemm stage: 5 conv1x1s over n in 2 chunks of 512.
    # Bias rides the augmented pos row; sigmoid is computed as tanh
    # (0.5*(1+tanh(x/2))) so ACT never switches ACT-table sets.
    q_sb = work.tile([128, N], F32R, tag="q")
    tk_sb = work.tile([128, N], F32, tag="tk")
    ck_sb = work.tile([128, N], F32, tag="ck")
    tv_sb = work.tile([128, N], F32, tag="tv")
    cv_sb = work.tile([128, N], F32, tag="cv")

    def gemm(dst, w3, kv, kind):
        wx, wp, wk = w3
        for j in (0, 1):
            sl = slice(512 * j, 512 * (j + 1))
            ps = psA.tile([128, 512], F32, tag="psA")
            nc.tensor.matmul(ps[:], r(wx[:]), r(xb[:, sl]), start=True, stop=False)
            last = wk is None
            nc.tensor.matmul(ps[:], r(wp[:]), r(pos[:, sl]), start=False, stop=last)
            if wk is not None:
                nc.tensor.matmul(ps[:], r(wk[:]), r(kv[:, sl]), start=False, stop=True)
            if kind == "relu_act":
                nc.scalar.activation(out=dst[:, sl], in_=ps[:], func=Act.Relu,
                                     scale=1.0)
            elif kind == "tanh":
                nc.scalar.activation(out=dst[:, sl], in_=ps[:], func=Act.Tanh,
                                     scale=0.5)
            else:  # relu on DVE
                nc.vector.tensor_scalar_max(dst[:, sl], ps[:], 0.0)

    gemm(q_sb, wq, None, "relu_act")
    gemm(tk_sb, wfk, kb, "tanh")
    gemm(ck_sb, wck, kb, "relu_dve")
    gemm(tv_sb, wfv, vb, "tanh")
    gemm(cv_sb, wcv, vb, "relu_dve")

    # ---- gated state updates: k_new = 0.5*(1+t_k)*k + c_k  (t = tanh(G/2))
    kn_sb = io.tile([128, N], F32, tag="kn")
    vn_sb = io.tile([128, N], F32, tag="vn")
    uk_sb = work.tile([128, N], F32, tag="uk")
    uv_sb = work.tile([128, N], F32, tag="uv")
    kbf = kb[:].bitcast(F32)
    vbf = vb[:].bitcast(F32)
    nc.gpsimd.scalar_tensor_tensor(out=uk_sb[:], in0=tk_sb[:], scalar=1.0,
                                   in1=kbf, op0=Alu.add, op1=Alu.mult)
    nc.gpsimd.scalar_tensor_tensor(out=uv_sb[:], in0=tv_sb[:], scalar=1.0,
                                   in1=vbf, op0=Alu.add, op1=Alu.mult)
    nc.vector.scalar_tensor_tensor(out=kn_sb[:], in0=uk_sb[:], scalar=0.5,
                                   in1=ck_sb[:], op0=Alu.mult, op1=Alu.add)
    nc.vector.scalar_tensor_tensor(out=vn_sb[:], in0=uv_sb[:], scalar=0.5,
                                   in1=cv_sb[:], op0=Alu.mult, op1=Alu.add)
    nc.sync.dma_start(out=d["kn"].ap()[b], in_=kn_sb[:])
    nc.sync.dma_start(out=d["vn"].ap()[b], in_=vn_sb[:])

    # ---- V^T via PE transposes (8 blocks of 128)
    vt_sb = work.tile([128, N], F32R, tag="vt")
    for g in (0, 1):
        pvt = psT.tile([128, 512], F32R, tag="psT")
        for i in range(4):
            c = 4 * g + i
            nc.tensor.transpose(pvt[:, 128 * i:128 * (i + 1)],
                                vb[:, 128 * c:128 * (c + 1)], ident[:])
        nc.vector.tensor_copy(vt_sb[:, 512 * g:512 * (g + 1)], pvt[:])

    # ---- S^T blocks + exp -> P^T (packed per m-block c, cols n in [128c, N))
    offs = []
    off = 0
    for c in range(8):
        offs.append(off)
        off += N - 128 * c
    pt_sb = work.tile([128, off], F32R, tag="pt")  # [128, 4608]

    for c in range(8):
        w = N - 128 * c
        n0 = 128 * c
        for s0 in range(0, w, 512):
            sw = min(512, w - s0)
            st = psA.tile([128, 512], F32, tag="psA")
            nc.tensor.matmul(st[:, :sw], r(kb[:, n0:n0 + 128]),
                             r(q_sb[:, n0 + s0:n0 + s0 + sw]), start=True, stop=True)
            nc.scalar.activation(out=pt_sb[:, offs[c] + s0:offs[c] + s0 + sw],
                                 in_=st[:, :sw], func=Act.Exp, scale=SCALE)
        # causal mask on the diagonal block: keep upper triangle (m <= n)
        nc.vector.tensor_mul(pt_sb[:, offs[c]:offs[c] + 128],
                             pt_sb[:, offs[c]:offs[c] + 128], triu[:])

    # ---- denominators: Z[n] = sum_m P^T[m, n] via ones-matmuls, then recip+bcast
    zps = psZ.tile([1, N], F32, tag="z")
    for j in (0, 1):
        cmax = min(7, (512 * (j + 1) - 1) // 128)
        for c in range(cmax + 1):
            n0 = max(512 * j, 128 * c)
            n1 = 512 * (j + 1)
            if n0 >= n1:
                continue
            sl_p = slice(offs[c] + n0 - 128 * c, offs[c] + n1 - 128 * c)
            nc.tensor.matmul(zps[:, n0:n1], r(ones[:]), r(pt_sb[:, sl_p]),
                             start=(c == 0), stop=(c == cmax))
    zrec = work.tile([1, N], F32, tag="zrec")
    nc.vector.reciprocal(zrec[:], zps[:])
    zb = work.tile([128, N], F32, tag="zb")
    nc.gpsimd.partition_broadcast(zb[:], zrec[:])

    # ---- PV: out[v, n] = sum_m V^T[m, v]^T-weighted P^T[m, n], then normalize
    o_sb = io.tile([128, N], F32, tag="o")
    for j in (0, 1):
        pv = psPV.tile([128, 512], F32, tag="pv")
        cmax = min(7, (512 * (j + 1) - 1) // 128)
        for c in range(cmax + 1):
            n0 = max(512 * j, 128 * c)
            n1 = 512 * (j + 1)
            if n0 >= n1:
                continue
            sl_p = slice(offs[c] + n0 - 128 * c, offs[c] + n1 - 128 * c)
            nc.tensor.matmul(pv[:, n0 - 512 * j:n1 - 512 * j],
                             r(vt_sb[:, 128 * c:128 * (c + 1)]), r(pt_sb[:, sl_p]),
                             start=(c == 0), stop=(c == cmax))
        sl = slice(512 * j, 512 * (j + 1))
        nc.vector.tensor_mul(o_sb[:, sl], pv[:], zb[:, sl])
    nc.sync.dma_start(out=d["o"].ap()[b], in_=o_sb[:])


# ------------------------------------------------------------- host wrapper

def make_in_maps(x, k, v, W_q, b_q, W_fk, b_fk, W_ck, b_ck, W_fv, b_fv,
                 W_cv, b_cv, W_fc=None, b_fc=None):
    """Shard full inputs into 8 per-core input maps."""
    x = np.asarray(x, dtype=np.float32).reshape(B, C, N)
    k = np.asarray(k, dtype=np.float32)
    v = np.asarray(v, dtype=np.float32)

    def wsplit(W, bias):
        W = np.asarray(W, dtype=np.float32)
        bias = np.asarray(bias, dtype=np.float32).reshape(1, -1)
        out = [np.ascontiguousarray(W[:, :C].T)]
        # pos-part augmented with the bias row (pairs with pos ones-row)
        out.append(np.ascontiguousarray(
            np.vstack([W[:, C:C + 2 * EMB].T, bias])))
        if W.shape[1] > C + 2 * EMB:
            out.append(np.ascontiguousarray(W[:, C + 2 * EMB:].T))
        return out

    wq = wsplit(W_q, b_q)
    wfk = wsplit(W_fk, b_fk)
    wck = wsplit(W_ck, b_ck)
    wfv = wsplit(W_fv, b_fv)
    wcv = wsplit(W_cv, b_cv)
    pos = np.vstack([_pos_embeddings(SIDE, EMB),
                     np.ones((1, N), np.float32)])
    triu = np.triu(np.ones((128, 128), np.float32))
    ident = np.eye(128, dtype=np.float32)

    shared = {
        "pos": pos, "triu": triu, "ident": ident,
        "wq_x": wq[0], "wq_p": wq[1],
        "wfk_x": wfk[0], "wfk_p": wfk[1], "wfk_k": wfk[2],
        "wck_x": wck[0], "wck_p": wck[1], "wck_k": wck[2],
        "wfv_x": wfv[0], "wfv_p": wfv[1], "wfv_k": wfv[2],
        "wcv_x": wcv[0], "wcv_p": wcv[1], "wcv_k": wcv[2],
        "ones": np.ones((128, 1), np.float32),
    }
    in_maps = []
    for c in range(N_CORES):
        sl = slice(c * BPC, (c + 1) * BPC)
        m = dict(shared)
        m["x"] = np.ascontiguousarray(x[sl])
        m["k"] = np.ascontiguousarray(k[sl])
        m["v"] = np.ascontiguousarray(v[sl])
        in_maps.append(m)
    return in_maps


def gather_results(results):
    o = np.concatenate([results[c]["o"] for c in range(N_CORES)], axis=0)
    kn = np.concatenate([results[c]["kn"] for c in range(N_CORES)], axis=0)
    vn = np.concatenate([results[c]["vn"] for c in range(N_CORES)], axis=0)
    return o.reshape(B, V, SIDE, SIDE), kn, vn


_CACHED_NC = None


def kernel(**inputs):
    global _CACHED_NC
    if _CACHED_NC is None:
        _CACHED_NC, _ = build_program(reps=1)
    in_maps = make_in_maps(**inputs)
    res = bass_utils.run_bass_kernel_spmd(
        _CACHED_NC, in_maps, core_ids=list(range(N_CORES)))
    return gather_results(res.results)


# revision 18
# speedup vs baseline: 57.8327x; 57.8327x over previous
"""Trainium2 Bass kernel for nn_AttentionBlockPersistent.

Computation (per batch b of 32):
  x0   = concat([x[b] (128ch), pos (32ch)])                 # [160, 1024]
  q    = relu(W_q @ x0 + b_q)                               # [128, 1024]
  f_k  = sigmoid(W_fk @ [x0; k] + b_fk);  c_k = relu(W_ck @ [x0; k] + b_ck)
  k_new = f_k * k + c_k        (same for v)
  S    = q^T K / sqrt(128)     causal-masked softmax rows   # [1024, 1024]
  out  = V @ softmax(S)^T                                   # [128, 1024]

Sharding: data-parallel over batch, 4 batches per core on 8 cores.
Weights/pos/masks replicated. All matmuls run as float32r (full-rate PE);
attention probabilities are computed in transposed layout [m, n] so the
PV matmul needs no transposes of P; the softmax denominator comes from a
ones-vector matmul and is applied via reciprocal + partition_broadcast.
"""

import os
import sys

for _p in ("/opt/trn_rl_repo", os.path.expanduser("~/.axon_site/_ro/trn_rl_repo")):
    if os.path.isdir(_p) and _p not in sys.path:
        sys.path.insert(0, _p)

import numpy as np

import concourse.bass as bass  # noqa: E402
import concourse.mybir as mybir  # noqa: E402
import concourse.tile as tile  # noqa: E402
from concourse import bacc, bass_utils  # noqa: E402

F32 = mybir.dt.float32
F32R = mybir.dt.float32r

N_CORES = 8
B = 32
BPC = B // N_CORES  # batches per core
C = 128
K = 128
V = 128
EMB = 16
SIDE = 32
N = SIDE * SIDE  # 1024
SCALE = 1.0 / float(np.sqrt(K))


# ---------------------------------------------------------------- host prep

def _pos_enc_table(side_len, emb_dim):
    tbl = np.array(
        [
            [p / np.power(10000.0, 2 * (j // 2) / emb_dim) for j in range(emb_dim)]
            if p != 0
            else np.zeros(emb_dim)
            for p in range(side_len)
        ]
    )
    tbl[1:, 0::2] = np.sin(tbl[1:, 0::2])
    tbl[1:, 1::2] = np.cos(tbl[1:, 1::2])
    return tbl.astype(np.float32)


def _pos_embeddings(side_len, emb_dim):
    t = _pos_enc_table(side_len, emb_dim)
    xe = np.broadcast_to(t.reshape(1, emb_dim, side_len, 1), (1, emb_dim, side_len, side_len))
    ye = np.broadcast_to(t.reshape(1, emb_dim, 1, side_len), (1, emb_dim, side_len, side_len))
    pe = np.concatenate([xe, ye], axis=1)
    return np.ascontiguousarray(pe.reshape(2 * emb_dim, side_len * side_len))  # [32, 1024]


# ------------------------------------------------------------- bass program

def build_program(reps: int = 1):
    """Build + compile the per-core program. Returns (nc, input_names)."""
    nc = bacc.Bacc("TRN2", target_bir_lowering=False, debug=False)

    dram_in = {}

    def din(name, shape, dt=F32R):
        t = nc.dram_tensor(name, list(shape), dt, kind="ExternalInput")
        dram_in[name] = t
        return t

    din("x", (BPC, C, N))
    din("k", (BPC, K, N))
    din("v", (BPC, V, N))
    din("pos", (2 * EMB, N))
    din("triu", (128, 128))
    din("ident", (128, 128))
    for nm in ("wq", "wfk", "wck", "wfv", "wcv"):
        din(nm + "_x", (C, K))
        if nm != "wq":
            din(nm + "_k", (K, K))
    for nm in ("wq", "wfk", "wck", "wfv", "wcv"):
        din(nm + "_p", (2 * EMB, K))
    for nm in ("bq", "bfk2", "bck", "bfv2", "bcv"):
        din(nm, (K, 1), dt=F32)
    din("ones", (128, 1))
    din("onesr", (1, 128))

    dram_in["o"] = nc.dram_tensor("o", [BPC, V, N], F32, kind="ExternalOutput")
    dram_in["kn"] = nc.dram_tensor("kn", [BPC, K, N], F32, kind="ExternalOutput")
    dram_in["vn"] = nc.dram_tensor("vn", [BPC, V, N], F32, kind="ExternalOutput")

    with tile.TileContext(nc) as tc:
        _emit(tc, nc, dram_in, reps)

    nc.compile()
    in_names = [n for n in dram_in if n not in ("o", "kn", "vn")]
    return nc, in_names


def _emit(tc, nc, d, reps):
    import contextlib

    ctx = contextlib.ExitStack()
    with ctx:
        consts = ctx.enter_context(tc.tile_pool(name="consts", bufs=1))
        io = ctx.enter_context(tc.tile_pool(name="io", bufs=2))
        work = ctx.enter_context(tc.tile_pool(name="work", bufs=2))
        psA = ctx.enter_context(tc.tile_pool(name="psA", bufs=3, space="PSUM"))
        psT = ctx.enter_context(tc.tile_pool(name="psT", bufs=1, space="PSUM"))
        psPV = ctx.enter_context(tc.tile_pool(name="psPV", bufs=2, space="PSUM"))
        psZ = ctx.enter_context(tc.tile_pool(name="psZ", bufs=1, space="PSUM"))

        # ---- load constants into SBUF once
        def cload(name, shape):
            t = consts.tile(list(shape), d[name].dtype, tag=name)
            nc.sync.dma_start(out=t[:], in_=d[name].ap())
            return t

        pos = cload("pos", (2 * EMB, N))
        triu = cload("triu", (128, 128))
        ident = cload("ident", (128, 128))
        wq_x = cload("wq_x", (C, K))
        wfk_x = cload("wfk_x", (C, K))
        wfk_k = cload("wfk_k", (K, K))
        wck_x = cload("wck_x", (C, K))
        wck_k = cload("wck_k", (K, K))
        wfv_x = cload("wfv_x", (C, V))
        wfv_v = cload("wfv_k", (V, V))
        wcv_x = cload("wcv_x", (C, V))
        wcv_v = cload("wcv_k", (V, V))
        wq_p = cload("wq_p", (2 * EMB, K))
        wfk_p = cload("wfk_p", (2 * EMB, K))
        wck_p = cload("wck_p", (2 * EMB, K))
        wfv_p = cload("wfv_p", (2 * EMB, K))
        wcv_p = cload("wcv_p", (2 * EMB, K))
        bq = cload("bq", (K, 1))
        bfk2 = cload("bfk2", (K, 1))
        bck = cload("bck", (K, 1))
        bfv2 = cload("bfv2", (K, 1))
        bcv = cload("bcv", (K, 1))
        ones = cload("ones", (128, 1))
        onesr = cload("onesr", (1, 128))

        def r(ap):
            return ap

        def body(_i=None):
            for b in range(BPC):
                _emit_batch(tc, nc, d, b, io, work, psA, psT, psPV, psZ, r, pos, triu,
                            ident, ones, onesr,
                            (wq_x, wq_p, None, bq),
                            (wfk_x, wfk_p, wfk_k, bfk2),
                            (wck_x, wck_p, wck_k, bck),
                            (wfv_x, wfv_p, wfv_v, bfv2),
                            (wcv_x, wcv_p, wcv_v, bcv))

        if reps == 1:
            body()
        else:
            with tc.For_i(0, reps, 1) as _i:
                body(_i)


def _emit_batch(tc, nc, d, b, io, work, psA, psT, psPV, psZ, r, pos, triu, ident, ones, onesr,
                wq, wfk, wck, wfv, wcv):
    Act = mybir.ActivationFunctionType
    Alu = mybir.AluOpType

    # ---- loads
    xb = io.tile([128, N], F32R, tag="xb")
    kb = io.tile([128, N], F32R, tag="kb")
    vb = io.tile([128, N], F32R, tag="vb")
    nc.sync.dma_start(out=xb[:], in_=d["x"].ap()[b])
    nc.sync.dma_start(out=kb[:], in_=d["k"].ap()[b])
    nc.sync.dma_start(out=vb[:], in_=d["v"].ap()[b])

    import kernel as _KM
    parts = _KM._PARTS

    # ---- gemm stage: 5 conv1x1s over n in 2 chunks of 512.
    # Bias rides the augmented pos row; sigmoid is computed as tanh
    # (0.5*(1+tanh(x/2))) so ACT never switches ACT-table sets.
    q_sb = work.tile([128, N], F32R, tag="q")
    tk_sb = work.tile([128, N], F32, tag="tk")
    ck_sb = work.tile([128, N], F32, tag="ck")
    tv_sb = work.tile([128, N], F32, tag="tv")
    cv_sb = work.tile([128, N], F32, tag="cv")

    specs = [("q", q_sb, wq), ("fk", tk_sb, wfk), ("ck", ck_sb, wck),
             ("fv", tv_sb, wfv), ("cv", cv_sb, wcv)]
    if parts not in ("all", "gemm", "dvegates"):
        specs = specs[:1]
    for nm, dst, w4 in specs:
        wx, wp, wk, bias = w4
        kv = kb if nm in ("fk", "ck") else vb
        for j in (0, 1):
            sl = slice(512 * j, 512 * (j + 1))
            ps = psA.tile([128, 512], F32, tag="psA")
            nc.tensor.matmul(ps[:], r(wx[:]), r(xb[:, sl]), start=True,
                             stop=False)
            nc.tensor.matmul(ps[:], r(wp[:]), r(pos[:, sl]), start=False,
                             stop=(wk is None))
            if wk is not None:
                nc.tensor.matmul(ps[:], r(wk[:]), r(kv[:, sl]), start=False,
                                 stop=True)
            if nm == "q":
                nc.scalar.activation(out=dst[:, sl], in_=ps[:], func=Act.Relu,
                                     bias=bias[:], scale=1.0)
            elif nm in ("fk", "fv"):
                nc.scalar.activation(out=dst[:, sl], in_=ps[:], func=Act.Tanh,
                                     bias=bias[:], scale=0.5)
            else:
                nc.vector.tensor_scalar(out=dst[:, sl], in0=ps[:],
                                        scalar1=bias[:], scalar2=0.0,
                                        op0=Alu.add, op1=Alu.max)

    # ---- gated state updates: k_new = 0.5*(1+t_k)*k + c_k  (t = tanh(G/2))
    kn_sb = io.tile([128, N], F32, tag="kn")
    vn_sb = io.tile([128, N], F32, tag="vn")
    if parts in ("all", "gemm", "dvegates"):
        _emit_gates(nc, d, b, work, kn_sb, vn_sb, tk_sb, tv_sb, ck_sb, cv_sb,
                    kb, vb, Alu)
    if parts in ("all", "attn", "nobc", "dvegates"):
        _emit_attn(tc, nc, d, b, io, work, psA, psT, psPV, psZ, r, triu, ident,
                   ones, onesr, xb, kb, vb, q_sb, Act, Alu)


def _emit_gates(nc, d, b, work, kn_sb, vn_sb, tk_sb, tv_sb, ck_sb, cv_sb,
                kb, vb, Alu):
    uk_sb = work.tile([128, N], F32, tag="uk")
    uv_sb = work.tile([128, N], F32, tag="uv")
    kbf = kb[:].bitcast(F32)
    vbf = vb[:].bitcast(F32)
    # f = 0.5*(1+tanh(x/2)) == sigmoid(x); k_new = f*k + c_k
    nc.vector.tensor_scalar(out=tk_sb[:], in0=tk_sb[:], scalar1=0.5,
                            scalar2=0.5, op0=Alu.mult, op1=Alu.add)
    nc.vector.tensor_scalar(out=tv_sb[:], in0=tv_sb[:], scalar1=0.5,
                            scalar2=0.5, op0=Alu.mult, op1=Alu.add)
    nc.vector.tensor_mul(uk_sb[:], tk_sb[:], kbf)
    nc.vector.tensor_mul(uv_sb[:], tv_sb[:], vbf)
    nc.vector.tensor_add(kn_sb[:], uk_sb[:], ck_sb[:])
    nc.vector.tensor_add(vn_sb[:], uv_sb[:], cv_sb[:])
    nc.sync.dma_start(out=d["kn"].ap()[b], in_=kn_sb[:])
    nc.sync.dma_start(out=d["vn"].ap()[b], in_=vn_sb[:])


def _emit_attn(tc, nc, d, b, io, work, psA, psT, psPV, psZ, r, triu, ident,
               ones, onesr, xb, kb, vb, q_sb, Act, Alu):
    # ---- V^T via PE transposes (8 blocks of 128)
    vt_sb = work.tile([128, N], F32R, tag="vt")
    for g in (0, 1):
        pvt = psT.tile([128, 512], F32R, tag="psT")
        for i in range(4):
            c = 4 * g + i
            nc.tensor.transpose(pvt[:, 128 * i:128 * (i + 1)],
                                vb[:, 128 * c:128 * (c + 1)], ident[:])
        nc.vector.tensor_copy(vt_sb[:, 512 * g:512 * (g + 1)], pvt[:])

    # ---- S^T blocks + exp -> P^T (packed per m-block c, cols n in [128c, N))
    offs = []
    off = 0
    for c in range(8):
        offs.append(off)
        off += N - 128 * c
    pt_sb = work.tile([128, off], F32R, tag="pt")  # [128, 4608]

    for c in range(8):
        w = N - 128 * c
        n0 = 128 * c
        for s0 in range(0, w, 512):
            sw = min(512, w - s0)
            st = psA.tile([128, 512], F32, tag="psA")
            nc.tensor.matmul(st[:, :sw], r(kb[:, n0:n0 + 128]),
                             r(q_sb[:, n0 + s0:n0 + s0 + sw]), start=True, stop=True)
            nc.scalar.activation(out=pt_sb[:, offs[c] + s0:offs[c] + s0 + sw],
                                 in_=st[:, :sw], func=Act.Exp, scale=SCALE)
        # causal mask on the diagonal block: keep upper triangle (m <= n)
        nc.vector.tensor_mul(pt_sb[:, offs[c]:offs[c] + 128],
                             pt_sb[:, offs[c]:offs[c] + 128], triu[:])

    # ---- denominators: Z[n] = sum_m P^T[m, n] via ones-matmuls, then recip+bcast
    zps = psZ.tile([1, N], F32, tag="z")
    for j in (0, 1):
        cmax = min(7, (512 * (j + 1) - 1) // 128)
        for c in range(cmax + 1):
            n0 = max(512 * j, 128 * c)
            n1 = 512 * (j + 1)
            if n0 >= n1:
                continue
            sl_p = slice(offs[c] + n0 - 128 * c, offs[c] + n1 - 128 * c)
            nc.tensor.matmul(zps[:, n0:n1], r(ones[:]), r(pt_sb[:, sl_p]),
                             start=(c == 0), stop=(c == cmax))
    import kernel as _KM2
    nobc = _KM2._PARTS == 'nobc'
    zrec = work.tile([1, N], F32R, tag="zrec")
    with nc.allow_low_precision(reason="f32r is fp32-width; rounding only"):
        nc.vector.reciprocal(zrec[:], zps[:])
    zb = work.tile([128, N], F32, tag="zb")
    if not nobc:
        for j in (0, 1):
            sl = slice(512 * j, 512 * (j + 1))
            zbp = psA.tile([128, 512], F32, tag="psA")
            nc.tensor.matmul(zbp[:], r(onesr[:]), r(zrec[:, sl]),
                             start=True, stop=True)
            nc.vector.tensor_copy(zb[:, sl], zbp[:])

    # ---- PV: out[v, n] = sum_m V^T[m, v]^T-weighted P^T[m, n], then normalize
    o_sb = io.tile([128, N], F32, tag="o")
    for j in (0, 1):
        pv = psPV.tile([128, 512], F32, tag="pv")
        cmax = min(7, (512 * (j + 1) - 1) // 128)
        for c in range(cmax + 1):
            n0 = max(512 * j, 128 * c)
            n1 = 512 * (j + 1)
            if n0 >= n1:
                continue
            sl_p = slice(offs[c] + n0 - 128 * c, offs[c] + n1 - 128 * c)
            nc.tensor.matmul(pv[:, n0 - 512 * j:n1 - 512 * j],
                             r(vt_sb[:, 128 * c:128 * (c + 1)]), r(pt_sb[:, sl_p]),
                             start=(c == 0), stop=(c == cmax))
        sl = slice(512 * j, 512 * (j + 1))
        if nobc:
            nc.vector.tensor_copy(o_sb[:, sl], pv[:])
        else:
            nc.vector.tensor_mul(o_sb[:, sl], pv[:], zb[:, sl])
    nc.sync.dma_start(out=d["o"].ap()[b], in_=o_sb[:])


# ------------------------------------------------------------- host wrapper

def make_in_maps(x, k, v, W_q, b_q, W_fk, b_fk, W_ck, b_ck, W_fv, b_fv,
                 W_cv, b_cv, W_fc=None, b_fc=None):
    """Shard full inputs into 8 per-core input maps."""
    x = np.asarray(x, dtype=np.float32).reshape(B, C, N)
    k = np.asarray(k, dtype=np.float32)
    v = np.asarray(v, dtype=np.float32)

    def wsplit(W):
        W = np.asarray(W, dtype=np.float32)
        out = [np.ascontiguousarray(W[:, :C].T),
               np.ascontiguousarray(W[:, C:C + 2 * EMB].T)]
        if W.shape[1] > C + 2 * EMB:
            out.append(np.ascontiguousarray(W[:, C + 2 * EMB:].T))
        return out

    wq = wsplit(W_q)
    wfk = wsplit(W_fk)
    wck = wsplit(W_ck)
    wfv = wsplit(W_fv)
    wcv = wsplit(W_cv)
    pos = _pos_embeddings(SIDE, EMB)
    triu = np.triu(np.ones((128, 128), np.float32))
    ident = np.eye(128, dtype=np.float32)

    shared = {
        "pos": pos, "triu": triu, "ident": ident,
        "wq_x": wq[0],
        "wfk_x": wfk[0], "wfk_k": wfk[2],
        "wck_x": wck[0], "wck_k": wck[2],
        "wfv_x": wfv[0], "wfv_k": wfv[2],
        "wcv_x": wcv[0], "wcv_k": wcv[2],
        "wq_p": wq[1], "wfk_p": wfk[1], "wck_p": wck[1],
        "wfv_p": wfv[1], "wcv_p": wcv[1],
        "bq": np.asarray(b_q, np.float32).reshape(K, 1),
        "bfk2": 0.5 * np.asarray(b_fk, np.float32).reshape(K, 1),
        "bck": np.asarray(b_ck, np.float32).reshape(K, 1),
        "bfv2": 0.5 * np.asarray(b_fv, np.float32).reshape(K, 1),
        "bcv": np.asarray(b_cv, np.float32).reshape(K, 1),
        "ones": np.ones((128, 1), np.float32),
        "onesr": np.ones((1, 128), np.float32),
    }
    in_maps = []
    for c in range(N_CORES):
        sl = slice(c * BPC, (c + 1) * BPC)
        m = dict(shared)
        m["x"] = np.ascontiguousarray(x[sl])
        m["k"] = np.ascontiguousarray(k[sl])
        m["v"] = np.ascontiguousarray(v[sl])
        in_maps.append(m)
    return in_maps


def gather_results(results):
    o = np.concatenate([results[c]["o"] for c in range(N_CORES)], axis=0)
    kn = np.concatenate([results[c]["kn"] for c in range(N_CORES)], axis=0)
    vn = np.concatenate([results[c]["vn"] for c in range(N_CORES)], axis=0)
    return o.reshape(B, V, SIDE, SIDE), kn, vn


_CACHED_NC = None
_PARTS = 'all'


def kernel(**inputs):
    global _CACHED_NC
    if _CACHED_NC is None:
        _CACHED_NC, _ = build_program(reps=1)
    in_maps = make_in_maps(**inputs)
    res = bass_utils.run_bass_kernel_spmd(
        _CACHED_NC, in_maps, core_ids=list(range(N_CORES)))
    return gather_results(res.results)
